# revision 40
# baseline (speedup 1.0000x reference)
"""Causal multi-head attention (B=1, S=4096, D=1024, H=16, HD=64) on 8 TRN2
NeuronCores, sharded 2 heads per core (tensor parallel).

Per core c (heads 2c, 2c+1; d-slice [128c, 128c+128)):
  - QT/KT/VT [128, 4096] = W_slice @ x.T via bf16 matmuls (xT streamed bf16).
  - V transposed back to [s, d] layout via PE identity-transpose, with a ones
    column appended (V65) so the ctx matmul also produces the softmax
    denominator as output column 64.
  - Flash-style causal attention with transposed scores ST[k, q]:
    exp on ScalarE (no running max: scaled scores are in [-4.1, 4.1] for this
    problem family; exp is safe in fp32). Scores for the two heads run on
    disjoint PE row-groups (partitions 0:64 / 64:128). Causal masking is
    multiplicative on the post-exp P tile (bf16, diagonal blocks only), which
    keeps DVE off the PE->ScalarE score->exp critical chain.
  - Context accumulated TRANSPOSED: per (k-tile, q-tile, head) a matmul with
    the P-chunk [128k x 128q] stationary and V65 [128k x 65] moving emits just
    65 PE rows (vs 128+ in q-streamed form) into a per-(head,q-tile) PSUM
    accumulator [128q x 65].  Stationary loads are free on PE.
  - Per-query softmax normalization fused into the PSUM->SBUF evacuation of
    the ctx accumulator (tensor_scalar mul by the reciprocal of column 64).
    Each (head, q-tile) accumulator is normalized / transposed / out-projected
    as soon as its diagonal k-tile retires, so the window epilogue overlaps
    the tail of the same window's exp stream instead of trailing the kernel.
  - ctx transposed back to [d, q] via PE (both heads packed into partitions
    0:64 / 64:128 of one PSUM tile), then y_c = ctx_c @ Wo[:, d-slice].T
    emitted per q-tile as [128, 1024] bf16.
Host: y = sum_c y_c + bo (fp32 accumulation on host).

The emission order pipelines phases: QKV projection for s-chunk w+2, the
V-transpose for chunk w+1 and the current window's per-q-tile epilogue are
issued as background subtasks between attention k-tiles so PE/DVE work hides
under the ScalarE exp stream (and vice versa).
"""

import numpy as np

import concourse.bacc as bacc
import concourse.mybir as mybir
import concourse.tile as tile
from concourse.bass import ds
from concourse.bass_utils import run_bass_kernel_spmd

P = 128
S = 4096
D = 1024
H = 16
HD = 64
NC = 8
WQ = 512            # query-window width
NW = S // WQ        # 8 windows
KTW = WQ // P       # 4 k-tiles per window width
IO = D // P         # 8 contraction tiles for projections
NKT = S // P        # 32 total k-tiles
SCALE = 1.0 / np.sqrt(HD)
WSC = 64.0      # fp8 weight pre-scale (undone on projection evacuation)

F32 = mybir.dt.float32
BF16 = mybir.dt.bfloat16

_CACHED_NC = None


def _build():
    nc = bacc.Bacc("TRN2", target_bir_lowering=False, debug=False, num_devices=NC)

    FP8 = mybir.dt.float8e4
    # x and W as fp8 main + residual pairs: W@x = w8@x8 + (wr@x8 + w8@xr)
    # (the dropped wr@xr term is ~0.4%^2); each projection then runs as
    # DoubleRow matmuls at 0.5 PE cycles/row
    xb = nc.dram_tensor("xb", [NW, P, IO, 2, WQ], FP8, kind="ExternalInput")
    wq8 = nc.dram_tensor("wq8", [P, IO, P], FP8, kind="ExternalInput")
    wqc = nc.dram_tensor("wqc", [P, IO, 2, P], FP8, kind="ExternalInput")
    wk8 = nc.dram_tensor("wk8", [P, IO, P], FP8, kind="ExternalInput")
    wkc = nc.dram_tensor("wkc", [P, IO, 2, P], FP8, kind="ExternalInput")
    wv8 = nc.dram_tensor("wv8", [P, IO, P], FP8, kind="ExternalInput")
    wvc = nc.dram_tensor("wvc", [P, IO, 2, P], FP8, kind="ExternalInput")
    wo = nc.dram_tensor("wo", [P, D], BF16, kind="ExternalInput")
    tri = nc.dram_tensor("tri", [P, P], BF16, kind="ExternalInput")
    ident = nc.dram_tensor("ident", [P, P], BF16, kind="ExternalInput")
    y = nc.dram_tensor("y", [S, D], BF16, kind="ExternalOutput")

    with tile.TileContext(nc) as tc:
        with (
            tc.tile_pool(name="const", bufs=1) as cpool,
            tc.tile_pool(name="bigs", bufs=1) as bigs,
            tc.tile_pool(name="xp", bufs=3) as xpool,
            tc.tile_pool(name="ptp", bufs=8) as ptpool,
            tc.tile_pool(name="ctxp", bufs=2) as ctxpool,
            tc.tile_pool(name="yp", bufs=4) as ypool,
            tc.tile_pool(name="ps_st", bufs=2, space="PSUM") as ps_st,
            tc.tile_pool(name="ps_ctx", bufs=1, space="PSUM") as ps_ctx,
            tc.tile_pool(name="ps_aux", bufs=2, space="PSUM") as ps_aux,
        ):
            # ---- latency-critical DMAs first: x chunk 0, Wq, Wk ----
            xts = {}
            xpend = {}

            def t_xt_dma(c8):
                def f():
                    xt = xpool.tile([P, IO, 2, WQ], FP8, tag="xt")
                    xts[c8] = xt
                    (nc.gpsimd if (c8 > 2 and c8 % 2 == 0) else nc.sync).dma_start(
                        xt[:], xb.ap()[c8])
                return f

            # chunk 0 x DMA split into io halves so the Q projection can
            # start as soon as the first half + Wq are resident
            xt0 = xpool.tile([P, IO, 2, WQ], FP8, tag="xt")
            xts[0] = xt0
            x0ap = xb.ap()[0]
            nc.sync.dma_start(xt0[:, 0 : IO // 2, :, :], x0ap[:, 0 : IO // 2, :, :])
            wq_sb = cpool.tile([P, IO, P], FP8, tag="wq")
            wqc_sb = cpool.tile([P, IO, 2, P], FP8, tag="wqc")
            wk_sb = cpool.tile([P, IO, P], FP8, tag="wk")
            wkc_sb = cpool.tile([P, IO, 2, P], FP8, tag="wkc")
            nc.sync.dma_start(wq_sb[:], wq8.ap())
            nc.sync.dma_start(wqc_sb[:], wqc.ap())
            nc.sync.dma_start(xt0[:, IO // 2 : IO, :, :], x0ap[:, IO // 2 : IO, :, :])
            nc.sync.dma_start(wk_sb[:], wk8.ap())
            nc.sync.dma_start(wkc_sb[:], wkc.ap())

            wv_sb = cpool.tile([P, IO, P], FP8, tag="wv")
            wvc_sb = cpool.tile([P, IO, 2, P], FP8, tag="wvc")
            nc.sync.dma_start(wv_sb[:], wv8.ap())
            nc.sync.dma_start(wvc_sb[:], wvc.ap())
            wo_sb = cpool.tile([P, D], BF16, tag="wo_sb")
            nc.sync.dma_start(wo_sb[:], wo.ap())
            tri_sb = cpool.tile([P, P], BF16, tag="tri")
            nc.sync.dma_start(tri_sb[:], tri.ap())
            id_sb = cpool.tile([P, P], BF16, tag="ident")
            nc.sync.dma_start(id_sb[:], ident.ap())
            warm_in = cpool.tile([P, 1], F32, tag="warm_in")
            nc.vector.memset(warm_in[:], 1.0)
            warm = cpool.tile([P, 1], F32, tag="warm")
            nc.scalar.activation(  # pull the exp table load off the hot path
                warm[:], warm_in[:],
                mybir.ActivationFunctionType.Exp, scale=1.0,
            )

            QT = bigs.tile([P, S], BF16, tag="QT")
            KT = bigs.tile([P, S], BF16, tag="KT")
            VT = bigs.tile([P, S], BF16, tag="VT")
            # V in [k, d] layout + ones column at 64 (denominator source)
            V65 = bigs.tile([P, 2, NKT, HD + 1], BF16, tag="V65")
            for h in (0, 1):
                nc.gpsimd.memset(V65[:, h, :, HD], 1.0)

            # ---- background task machinery ----
            # bg_pre: front-loaded projection/V-transpose/DMA tasks with
            # per-task deadline steps (popped urgently at the deadline,
            # opportunistically at a steady rate before it).
            # bg_epi: per-q-tile window epilogues, popped eagerly.
            from collections import deque
            bg_pre = []   # (deadline_step, seq, fn) heap-free sorted list
            bg_epi = deque()

            def pace(i, opp=0, epi=4):
                n = 0
                while bg_epi and n < epi:
                    bg_epi.popleft()()
                    n += 1
                n = 0
                while bg_pre and (bg_pre[0][0] <= i or n < opp):
                    bg_pre.pop(0)[2]()
                    n += 1

            DR = mybir.MatmulPerfMode.DoubleRow

            def t_proj(c8, wsb, wcsb, dest, c0=0, c1=WQ, eng=None):
                # two halves aligned to the x-chunk io-halves: main term
                # (w8 @ x8) as two-k-tile DoubleRow matmuls, cross terms
                # (wr @ x8 + w8 @ xr) as one DoubleRow per k-tile
                state = {}

                def half(lo):
                    def f():
                        if lo == 0:
                            ps = ps_aux.tile([P, WQ], F32, tag="aux")
                            state["ps"] = ps
                        ps = state["ps"]
                        for iop in range(lo // 2, lo // 2 + IO // 4):
                            nc.tensor.matmul(
                                ps[:, c0:c1],
                                wsb[:, 2 * iop : 2 * iop + 2, :],
                                xts[c8][:, 2 * iop : 2 * iop + 2, 0, c0:c1],
                                start=(iop == 0), stop=False, perf_mode=DR,
                            )
                        for io in range(lo, lo + IO // 2):
                            nc.tensor.matmul(
                                ps[:, c0:c1],
                                wcsb[:, io, :, :],
                                xts[c8][:, io, :, c0:c1],
                                start=False, stop=(io == IO - 1), perf_mode=DR,
                            )
                        if lo > 0:
                            if eng == "scalar":
                                nc.scalar.mul(
                                    dest[:, ds(c8 * WQ + c0, c1 - c0)],
                                    ps[:, c0:c1], 1.0 / WSC)
                            else:
                                nc.vector.tensor_scalar_mul(
                                    dest[:, ds(c8 * WQ + c0, c1 - c0)],
                                    ps[:, c0:c1], 1.0 / WSC)
                    return f
                return half(0), half(IO // 2)

            def t_vtrans(t):
                def f():
                    tp = ps_aux.tile([P, WQ], F32, tag="aux")
                    tpb = tp[:, 0:P].bitcast(BF16)[:, 0:P]
                    nc.tensor.transpose(tpb[:], VT[:, ds(t * P, P)], id_sb[:])
                    nc.vector.tensor_copy(V65[:, 0, t, 0:HD], tpb[:, 0:HD])
                    nc.vector.tensor_copy(V65[:, 1, t, 0:HD], tpb[:, HD:P])
                return f

            # ---- per-(head, q-tile) epilogue ----
            def t_norm(ctx_ps, ctxn, rcp, h, qt, eng=None):
                def f():
                    nc.vector.reciprocal(
                        rcp[:, 4 * h + qt : 4 * h + qt + 1],
                        ctx_ps[h][:, qt, HD : HD + 1],
                    )
                    if eng == "scalar":
                        nc.scalar.activation(
                            ctxn[:, qt, h, :], ctx_ps[h][:, qt, 0:HD],
                            mybir.ActivationFunctionType.Copy,
                            scale=rcp[:, 4 * h + qt : 4 * h + qt + 1],
                        )
                    else:
                        nc.vector.tensor_scalar_mul(
                            ctxn[:, qt, h, :],
                            ctx_ps[h][:, qt, 0:HD],
                            rcp[:, 4 * h + qt : 4 * h + qt + 1],
                        )
                return f

            def t_ctrans(ctxn, ctxsb, qt, eng=None):
                def f():
                    tp = ps_aux.tile([P, WQ], F32, tag="aux")
                    tpb = tp[:, 0:P].bitcast(BF16)[:, 0:P]
                    nc.tensor.transpose(tpb[:], ctxn[:, qt], id_sb[:])
                    if eng == "scalar":
                        nc.scalar.copy(ctxsb[:, qt, :], tpb[:])
                    else:
                        nc.vector.tensor_copy(ctxsb[:, qt, :], tpb[:])
                return f

            def t_outproj(w, ctxsb, qt, eng=None):
                state = {}

                def mk(oc):
                    def f():
                        if oc == 0:
                            ysb = ypool.tile([P, D], BF16, tag="ysb")
                            state["ysb"] = ysb
                        ysb = state["ysb"]
                        yps = ps_aux.tile([P, WQ], F32, tag="aux")
                        nc.tensor.matmul(
                            yps[:],
                            ctxsb[:, qt, :], wo_sb[:, ds(oc * WQ, WQ)],
                            start=True, stop=True,
                        )
                        if eng == "scalar" or (eng == "split" and oc == 0):
                            nc.scalar.copy(ysb[:, ds(oc * WQ, WQ)], yps[:])
                        else:
                            nc.vector.tensor_copy(ysb[:, ds(oc * WQ, WQ)], yps[:])
                        nc.sync.dma_start(
                            y.ap()[ds(w * WQ + qt * P, P), ds(oc * WQ, WQ)],
                            ysb[:, ds(oc * WQ, WQ)])
                    return f
                return mk(0), mk(1)

            # ctx emission runs a few k-tiles behind the score/exp stream and
            # the score matmul for step i+1 is emitted BEFORE the exp for
            # step i, so the next exp's input is always already in the PE
            # queue ahead of the ctx burst (ScalarE never waits on scores).
            pending = deque()  # entries: (w, emit_fn, kt, pt)
            wstate = {}        # w -> (ctx_ps, ctxn, ctxsb, rcp)
            sts = {}           # (w, kt) -> st tile

            def get_wstate(w):
                if w not in wstate:
                    ctx_a = ps_ctx.tile([P, KTW, HD + 1], F32, tag="ctx0")
                    ctx_b = ps_ctx.tile([P, KTW, HD + 1], F32, tag="ctx1")
                    # explicit zero of the accumulator banks: a framework-
                    # visible write ordered after the previous window's norm
                    # reads (the matmuls below accumulate with start=False,
                    # so no bank-wide pending-zero side effect races ahead)
                    nc.vector.memset(ctx_a[:], 0.0)
                    nc.vector.memset(ctx_b[:], 0.0)
                    ctxn = ctxpool.tile([P, KTW, 2, HD], BF16, tag="ctxn")
                    ctxsb = ctxpool.tile([P, KTW, P], BF16, tag="ctxsb")
                    rcp = ctxpool.tile([P, 2 * KTW], F32, tag="rcp")
                    wstate[w] = ([ctx_a, ctx_b], ctxn, ctxsb, rcp)
                return wstate[w]

            def make_emit_ctx(w):
                ctx_ps, ctxn, ctxsb, rcp = get_wstate(w)

                def emit_ctx(kt, pt):
                    jo = kt - KTW * w
                    if jo >= 0:
                        # diagonal block: multiplicative causal mask on the
                        # post-exp P tile (bf16); only q-tile jo is partial
                        meng = nc.gpsimd if w == NW - 1 else nc.vector
                        for h in (0, 1):
                            meng.tensor_mul(
                                pt[:, h, ds(P * jo, P)],
                                pt[:, h, ds(P * jo, P)], tri_sb[:],
                            )
                    # ONE psum accumulation group per head-bank per window:
                    # start on the very first matmul (its start marks the
                    # whole 2KB bank pending-zero, so every q-tile region
                    # starts from zero), stop on the very last; interior
                    # matmuls accumulate (first touch of a pending byte
                    # overwrites).
                    nkt = KTW * (w + 1)
                    for h in (0, 1):
                        for qt in range(max(0, jo), KTW):
                            nc.tensor.matmul(
                                ctx_ps[h][:, qt, :],
                                pt[:, h, ds(qt * P, P)], V65[:, h, kt, :],
                                start=False,
                                stop=(kt == nkt - 1 and qt == KTW - 1),
                                skip_group_check=True,
                            )
                    if jo >= 0:
                        # accumulator (h, jo) just retired: queue its epilogue
                        qt = jo
                        lastw = w == NW - 1
                        last = lastw and qt == KTW - 1
                        bg_epi.append(t_norm(
                            ctx_ps, ctxn, rcp, 0, qt,
                            eng="scalar" if last else None))
                        bg_epi.append(t_norm(ctx_ps, ctxn, rcp, 1, qt))
                        bg_epi.append(t_ctrans(
                            ctxn, ctxsb, qt, eng="scalar" if last else None))
                        bg_epi.extend(t_outproj(
                            w, ctxsb, qt, eng="split" if lastw else None))
                return emit_ctx

            emitters = {}

            def emit_scores(w, kt):
                jo = kt - KTW * w
                soff = P * jo if jo > 0 else 0
                st = ps_st.tile([P, 2, WQ], F32, tag="st")
                for h in (0, 1):
                    ph = ds(HD * h, HD)
                    nc.tensor.matmul(
                        st[:, h, soff:WQ],
                        KT[ph, ds(kt * P, P)], QT[ph, ds(w * WQ + soff, WQ - soff)],
                        start=True, stop=True,
                        tile_position=(HD * h, 0),
                    )
                sts[(w, kt)] = st

            def run_attention():
                steps = [(w, kt) for w in range(NW) for kt in range(KTW * (w + 1))]
                # urgent-pop the rest of chunk 0's K/V projections BEFORE the
                # one-ahead score stream starts reading them (engine program
                # order must put writers before readers)
                pace(-1)
                scored = 0  # steps[0] scores were emitted in the prologue
                for i, (w, kt) in enumerate(steps):
                    if w not in emitters:
                        emitters[w] = make_emit_ctx(w)
                    nkt = KTW * (w + 1)
                    jo = kt - KTW * w
                    soff = P * jo if jo > 0 else 0
                    if scored < min(i + 1, len(steps) - 1):
                        scored += 1
                        emit_scores(*steps[scored])
                    st = sts.pop((w, kt))
                    pt = ptpool.tile([P, 2, WQ], BF16, tag="pt")
                    nc.scalar.activation(
                        pt[:, :, soff:WQ], st[:, :, soff:WQ],
                        mybir.ActivationFunctionType.Exp, scale=SCALE,
                    )
                    pending.append((w, emitters[w], kt, pt))
                    # drain carried ctx from the previous window first;
                    # near the end of the LAST window drain eagerly so the
                    # per-q-tile epilogues overlap the exp tail
                    stag = 0 if (kt >= nkt - 3 or w == NW - 1) else 3
                    drained = 0
                    while pending and pending[0][0] != w and drained < 2:
                        _, fn, *a_ = pending.popleft()
                        fn(*a_)
                        drained += 1
                    while (pending and pending[0][0] == w
                           and len(pending) > stag and drained < 4):
                        _, fn, *a_ = pending.popleft()
                        fn(*a_)
                        drained += 1
                    pace(i, epi=6 if w == NW - 1 else (4 if (jo < 1 and kt >= 3) else 1))

            # ---- software-pipelined emission ----
            # PE warm-up: dummy matmuls on (uninitialized) SBUF ramp the PE
            # p-state to full clock while the first x/weight DMAs land
            warm_mm = cpool.tile([P, WQ], BF16, tag="warm_mm")
            nc.vector.memset(warm_mm[:], 0.0)
            warm_ps = ps_st.tile([P, 2, WQ], F32, tag="st")
            NWARM = 9
            for i in range(NWARM):
                nc.tensor.matmul(
                    warm_ps[:, 0, :], warm_mm[:, 0:P], warm_mm[:],
                    start=(i == 0), stop=(i == NWARM - 1), skip_group_check=True,
                )
            # prologue: only what window 0's first scores need (Q chunk 0 and
            # the first k-tile column block of K); the rest rides in bg
            for _f in t_proj(0, wq_sb, wqc_sb, QT):
                _f()
            for _f in t_proj(0, wk_sb, wkc_sb, KT, 0, P):
                _f()
            emit_scores(0, 0)
            t_xt_dma(1)()
            t_xt_dma(2)()

            def SW(w):
                return 2 * w * (w + 1)  # step index of window w's first k-tile

            pre = []  # (deadline, fn) in dependency order

            ka, kb = t_proj(0, wk_sb, wkc_sb, KT, P, WQ, eng="scalar")
            pre += [(-2, ka), (-2, kb)]
            va, vb = t_proj(0, wv_sb, wvc_sb, VT, eng="scalar")
            pre += [(-1, va), (-1, vb)]
            for t in range(KTW):
                pre.append((max(-1, t - 2), t_vtrans(t)))
            qa, qb = t_proj(1, wq_sb, wqc_sb, QT)
            pre += [(SW(1) - 4, qa), (SW(1) - 3, qb)]
            ka, kb = t_proj(1, wk_sb, wkc_sb, KT)
            pre += [(SW(1) - 1, ka), (SW(1), kb)]
            va, vb = t_proj(1, wv_sb, wvc_sb, VT)
            pre += [(SW(1) + KTW - 10, va), (SW(1) + KTW - 9, vb)]
            for t in range(KTW, 2 * KTW):
                pre.append((SW(1) + t - 4, t_vtrans(t)))
            for c in range(2, NW):
                if c + 1 < NW:
                    pre.append((SW(c) - 14, t_xt_dma(c + 1)))
                qa, qb = t_proj(c, wq_sb, wqc_sb, QT)
                pre += [(SW(c) - 8, qa), (SW(c) - 7, qb)]
                ka, kb = t_proj(c, wk_sb, wkc_sb, KT)
                kd = SW(c) + KTW * c - (9 if c > 2 else 6)
                pre += [(kd, ka), (kd + 1, kb)]
                va, vb = t_proj(c, wv_sb, wvc_sb, VT)
                vd = SW(c) + KTW * c - (12 if c > 2 else 8)
                pre += [(vd, va), (vd + 1, vb)]
                for t in range(KTW * c, KTW * (c + 1)):
                    pre.append((SW(c) + t - 4, t_vtrans(t)))
            for seq, (d, fn) in enumerate(pre):
                bg_pre.append((d, seq, fn))
            bg_pre.sort()

            run_attention()

            # epilogue: flush the ctx backlog (which queues the last window's
            # per-q-tile epilogues), then drain both queues
            while pending:
                _, fn, *a_ = pending.popleft()
                fn(*a_)
            while bg_pre:
                bg_pre.pop(0)[2]()
            while bg_epi:
                bg_epi.popleft()()

    nc.compile()
    return nc


def _get_nc():
    global _CACHED_NC
    if _CACHED_NC is None:
        _CACHED_NC = _build()
    return _CACHED_NC


def kernel(x, Wq, Wk, Wv, Wo, bo):
    import ml_dtypes

    x = np.asarray(x, dtype=np.float32)
    Wq = np.asarray(Wq, dtype=np.float32)
    Wk = np.asarray(Wk, dtype=np.float32)
    Wv = np.asarray(Wv, dtype=np.float32)
    Wo = np.asarray(Wo, dtype=np.float32)
    bo = np.asarray(bo, dtype=np.float32)

    bf = ml_dtypes.bfloat16
    e4 = ml_dtypes.float8_e4m3
    xT = np.ascontiguousarray(x.reshape(S, D).T)
    x8 = xT.astype(e4)
    xr = (xT - x8.astype(np.float32)).astype(e4)
    # [NW, P, IO, 2, WQ]: partition-first, contiguous per partition row
    def xprep(a):
        return a.reshape(IO, P, NW, WQ).transpose(2, 1, 0, 3)
    xb = np.ascontiguousarray(
        np.stack([xprep(x8), xprep(xr)], axis=3))
    col = np.arange(P)
    # tri[k, q] = 1 where q >= k (valid), 0 above the diagonal
    tri = (col[None, :] >= col[:, None]).astype(bf)
    ident = np.eye(P, dtype=np.float32).astype(bf)

    def wsplit(W, dsl):
        wT = np.ascontiguousarray(W[dsl, :].T) * WSC
        w8 = wT.astype(e4)
        wr = (wT - w8.astype(np.float32)).astype(e4)
        # [P, IO, P] / [P, IO, 2, P]: partition-first
        w8p = w8.reshape(IO, P, P).transpose(1, 0, 2)
        wrp = wr.reshape(IO, P, P).transpose(1, 0, 2)
        wc = np.ascontiguousarray(np.stack([wrp, w8p], axis=2))
        return np.ascontiguousarray(w8p), wc

    in_maps = []
    for c in range(NC):
        dsl = slice(P * c, P * (c + 1))
        q8, qc = wsplit(Wq, dsl)
        k8, kc = wsplit(Wk, dsl)
        v8, vc = wsplit(Wv, dsl)
        in_maps.append({
            "xb": xb,
            "wq8": q8, "wqc": qc,
            "wk8": k8, "wkc": kc,
            "wv8": v8, "wvc": vc,
            "wo": np.ascontiguousarray(Wo[:, dsl].T).astype(bf),
            "tri": tri,
            "ident": ident,
        })

    nc = _get_nc()
    res = run_bass_kernel_spmd(nc, in_maps, core_ids=list(range(NC)))
    out = np.zeros((S, D), dtype=np.float32)
    for c in range(NC):
        out += res.results[c]["y"].astype(np.float32)
    out += bo[None, :]
    return out.reshape(1, S, D)


# revision 41
# speedup vs baseline: 1.0006x; 1.0006x over previous
"""Causal multi-head attention (B=1, S=4096, D=1024, H=16, HD=64) on 8 TRN2
NeuronCores, sharded 2 heads per core (tensor parallel).

Per core c (heads 2c, 2c+1; d-slice [128c, 128c+128)):
  - QT/KT/VT [128, 4096] = W_slice @ x.T via bf16 matmuls (xT streamed bf16).
  - V transposed back to [s, d] layout via PE identity-transpose, with a ones
    column appended (V65) so the ctx matmul also produces the softmax
    denominator as output column 64.
  - Flash-style causal attention with transposed scores ST[k, q]:
    exp on ScalarE (no running max: scaled scores are in [-4.1, 4.1] for this
    problem family; exp is safe in fp32). Scores for the two heads run on
    disjoint PE row-groups (partitions 0:64 / 64:128). Causal masking is
    multiplicative on the post-exp P tile (bf16, diagonal blocks only), which
    keeps DVE off the PE->ScalarE score->exp critical chain.
  - Context accumulated TRANSPOSED: per (k-tile, q-tile, head) a matmul with
    the P-chunk [128k x 128q] stationary and V65 [128k x 65] moving emits just
    65 PE rows (vs 128+ in q-streamed form) into a per-(head,q-tile) PSUM
    accumulator [128q x 65].  Stationary loads are free on PE.
  - Per-query softmax normalization fused into the PSUM->SBUF evacuation of
    the ctx accumulator (tensor_scalar mul by the reciprocal of column 64).
    Each (head, q-tile) accumulator is normalized / transposed / out-projected
    as soon as its diagonal k-tile retires, so the window epilogue overlaps
    the tail of the same window's exp stream instead of trailing the kernel.
  - ctx transposed back to [d, q] via PE (both heads packed into partitions
    0:64 / 64:128 of one PSUM tile), then y_c = ctx_c @ Wo[:, d-slice].T
    emitted per q-tile as [128, 1024] bf16.
Host: y = sum_c y_c + bo (fp32 accumulation on host).

The emission order pipelines phases: QKV projection for s-chunk w+2, the
V-transpose for chunk w+1 and the current window's per-q-tile epilogue are
issued as background subtasks between attention k-tiles so PE/DVE work hides
under the ScalarE exp stream (and vice versa).
"""

import numpy as np

import concourse.bacc as bacc
import concourse.mybir as mybir
import concourse.tile as tile
from concourse.bass import ds
from concourse.bass_utils import run_bass_kernel_spmd

P = 128
S = 4096
D = 1024
H = 16
HD = 64
NC = 8
WQ = 512            # query-window width
NW = S // WQ        # 8 windows
KTW = WQ // P       # 4 k-tiles per window width
IO = D // P         # 8 contraction tiles for projections
NKT = S // P        # 32 total k-tiles
SCALE = 1.0 / np.sqrt(HD)
WSC = 64.0      # fp8 weight pre-scale (undone on projection evacuation)

F32 = mybir.dt.float32
BF16 = mybir.dt.bfloat16

_CACHED_NC = None


def _build():
    nc = bacc.Bacc("TRN2", target_bir_lowering=False, debug=False, num_devices=NC)

    FP8 = mybir.dt.float8e4
    # x and W as fp8 main + residual pairs: W@x = w8@x8 + (wr@x8 + w8@xr)
    # (the dropped wr@xr term is ~0.4%^2); each projection then runs as
    # DoubleRow matmuls at 0.5 PE cycles/row
    xb = nc.dram_tensor("xb", [NW, P, IO, 2, WQ], FP8, kind="ExternalInput")
    wq8 = nc.dram_tensor("wq8", [P, IO, P], FP8, kind="ExternalInput")
    wqc = nc.dram_tensor("wqc", [P, IO, 2, P], FP8, kind="ExternalInput")
    wk8 = nc.dram_tensor("wk8", [P, IO, P], FP8, kind="ExternalInput")
    wkc = nc.dram_tensor("wkc", [P, IO, 2, P], FP8, kind="ExternalInput")
    wv8 = nc.dram_tensor("wv8", [P, IO, P], FP8, kind="ExternalInput")
    wvc = nc.dram_tensor("wvc", [P, IO, 2, P], FP8, kind="ExternalInput")
    wo = nc.dram_tensor("wo", [P, D], BF16, kind="ExternalInput")
    tri = nc.dram_tensor("tri", [P, P], BF16, kind="ExternalInput")
    ident = nc.dram_tensor("ident", [P, P], BF16, kind="ExternalInput")
    y = nc.dram_tensor("y", [S, D], BF16, kind="ExternalOutput")

    with tile.TileContext(nc) as tc:
        with (
            tc.tile_pool(name="const", bufs=1) as cpool,
            tc.tile_pool(name="bigs", bufs=1) as bigs,
            tc.tile_pool(name="xp", bufs=3) as xpool,
            tc.tile_pool(name="ptp", bufs=8) as ptpool,
            tc.tile_pool(name="ctxp", bufs=2) as ctxpool,
            tc.tile_pool(name="yp", bufs=6) as ypool,
            tc.tile_pool(name="ps_st", bufs=2, space="PSUM") as ps_st,
            tc.tile_pool(name="ps_ctx", bufs=1, space="PSUM") as ps_ctx,
            tc.tile_pool(name="ps_aux", bufs=2, space="PSUM") as ps_aux,
        ):
            # ---- latency-critical DMAs first: x chunk 0, Wq, Wk ----
            xts = {}
            xpend = {}

            def t_xt_dma(c8):
                def f():
                    xt = xpool.tile([P, IO, 2, WQ], FP8, tag="xt")
                    xts[c8] = xt
                    (nc.gpsimd if (c8 > 2 and c8 % 2 == 0) else nc.sync).dma_start(
                        xt[:], xb.ap()[c8])
                return f

            # chunk 0 x DMA split into io halves so the Q projection can
            # start as soon as the first half + Wq are resident
            xt0 = xpool.tile([P, IO, 2, WQ], FP8, tag="xt")
            xts[0] = xt0
            x0ap = xb.ap()[0]
            nc.sync.dma_start(xt0[:, 0 : IO // 2, :, :], x0ap[:, 0 : IO // 2, :, :])
            wq_sb = cpool.tile([P, IO, P], FP8, tag="wq")
            wqc_sb = cpool.tile([P, IO, 2, P], FP8, tag="wqc")
            wk_sb = cpool.tile([P, IO, P], FP8, tag="wk")
            wkc_sb = cpool.tile([P, IO, 2, P], FP8, tag="wkc")
            nc.sync.dma_start(wq_sb[:], wq8.ap())
            nc.sync.dma_start(wqc_sb[:], wqc.ap())
            nc.sync.dma_start(xt0[:, IO // 2 : IO, :, :], x0ap[:, IO // 2 : IO, :, :])
            nc.sync.dma_start(wk_sb[:], wk8.ap())
            nc.sync.dma_start(wkc_sb[:], wkc.ap())

            wv_sb = cpool.tile([P, IO, P], FP8, tag="wv")
            wvc_sb = cpool.tile([P, IO, 2, P], FP8, tag="wvc")
            nc.sync.dma_start(wv_sb[:], wv8.ap())
            nc.sync.dma_start(wvc_sb[:], wvc.ap())
            wo_sb = cpool.tile([P, D], BF16, tag="wo_sb")
            nc.sync.dma_start(wo_sb[:], wo.ap())
            tri_sb = cpool.tile([P, P], BF16, tag="tri")
            nc.sync.dma_start(tri_sb[:], tri.ap())
            id_sb = cpool.tile([P, P], BF16, tag="ident")
            nc.sync.dma_start(id_sb[:], ident.ap())
            warm_in = cpool.tile([P, 1], F32, tag="warm_in")
            nc.vector.memset(warm_in[:], 1.0)
            warm = cpool.tile([P, 1], F32, tag="warm")
            nc.scalar.activation(  # pull the exp table load off the hot path
                warm[:], warm_in[:],
                mybir.ActivationFunctionType.Exp, scale=1.0,
            )

            QT = bigs.tile([P, S], BF16, tag="QT")
            KT = bigs.tile([P, S], BF16, tag="KT")
            VT = bigs.tile([P, S], BF16, tag="VT")
            # V in [k, d] layout + ones column at 64 (denominator source)
            V65 = bigs.tile([P, 2, NKT, HD + 1], BF16, tag="V65")
            for h in (0, 1):
                nc.gpsimd.memset(V65[:, h, :, HD], 1.0)

            # ---- background task machinery ----
            # bg_pre: front-loaded projection/V-transpose/DMA tasks with
            # per-task deadline steps (popped urgently at the deadline,
            # opportunistically at a steady rate before it).
            # bg_epi: per-q-tile window epilogues, popped eagerly.
            from collections import deque
            bg_pre = []   # (deadline_step, seq, fn) heap-free sorted list
            bg_epi = deque()

            def pace(i, opp=0, epi=4):
                n = 0
                while bg_epi and n < epi:
                    bg_epi.popleft()()
                    n += 1
                n = 0
                while bg_pre and (bg_pre[0][0] <= i or n < opp):
                    bg_pre.pop(0)[2]()
                    n += 1

            DR = mybir.MatmulPerfMode.DoubleRow

            def t_proj(c8, wsb, wcsb, dest, c0=0, c1=WQ, eng=None):
                # two halves aligned to the x-chunk io-halves: main term
                # (w8 @ x8) as two-k-tile DoubleRow matmuls, cross terms
                # (wr @ x8 + w8 @ xr) as one DoubleRow per k-tile
                state = {}

                def half(lo):
                    def f():
                        if lo == 0:
                            ps = ps_aux.tile([P, WQ], F32, tag="aux")
                            state["ps"] = ps
                        ps = state["ps"]
                        for iop in range(lo // 2, lo // 2 + IO // 4):
                            nc.tensor.matmul(
                                ps[:, c0:c1],
                                wsb[:, 2 * iop : 2 * iop + 2, :],
                                xts[c8][:, 2 * iop : 2 * iop + 2, 0, c0:c1],
                                start=(iop == 0), stop=False, perf_mode=DR,
                            )
                        for io in range(lo, lo + IO // 2):
                            nc.tensor.matmul(
                                ps[:, c0:c1],
                                wcsb[:, io, :, :],
                                xts[c8][:, io, :, c0:c1],
                                start=False, stop=(io == IO - 1), perf_mode=DR,
                            )
                        if lo > 0:
                            if eng == "scalar":
                                nc.scalar.mul(
                                    dest[:, ds(c8 * WQ + c0, c1 - c0)],
                                    ps[:, c0:c1], 1.0 / WSC)
                            else:
                                nc.vector.tensor_scalar_mul(
                                    dest[:, ds(c8 * WQ + c0, c1 - c0)],
                                    ps[:, c0:c1], 1.0 / WSC)
                    return f
                return half(0), half(IO // 2)

            def t_vtrans(t):
                def f():
                    tp = ps_aux.tile([P, WQ], F32, tag="aux")
                    tpb = tp[:, 0:P].bitcast(BF16)[:, 0:P]
                    nc.tensor.transpose(tpb[:], VT[:, ds(t * P, P)], id_sb[:])
                    nc.vector.tensor_copy(V65[:, 0, t, 0:HD], tpb[:, 0:HD])
                    nc.vector.tensor_copy(V65[:, 1, t, 0:HD], tpb[:, HD:P])
                return f

            # ---- per-(head, q-tile) epilogue ----
            def t_norm(ctx_ps, ctxn, rcp, h, qt, eng=None):
                def f():
                    nc.vector.reciprocal(
                        rcp[:, 4 * h + qt : 4 * h + qt + 1],
                        ctx_ps[h][:, qt, HD : HD + 1],
                    )
                    if eng == "scalar":
                        nc.scalar.activation(
                            ctxn[:, qt, h, :], ctx_ps[h][:, qt, 0:HD],
                            mybir.ActivationFunctionType.Copy,
                            scale=rcp[:, 4 * h + qt : 4 * h + qt + 1],
                        )
                    else:
                        nc.vector.tensor_scalar_mul(
                            ctxn[:, qt, h, :],
                            ctx_ps[h][:, qt, 0:HD],
                            rcp[:, 4 * h + qt : 4 * h + qt + 1],
                        )
                return f

            def t_ctrans(ctxn, ctxsb, qt, eng=None):
                def f():
                    tp = ps_aux.tile([P, WQ], F32, tag="aux")
                    tpb = tp[:, 0:P].bitcast(BF16)[:, 0:P]
                    nc.tensor.transpose(tpb[:], ctxn[:, qt], id_sb[:])
                    if eng == "scalar":
                        nc.scalar.copy(ctxsb[:, qt, :], tpb[:])
                    else:
                        nc.vector.tensor_copy(ctxsb[:, qt, :], tpb[:])
                return f

            def t_outproj(w, ctxsb, qt, eng=None):
                state = {}

                def mk(oc):
                    def f():
                        if oc == 0:
                            ysb = ypool.tile([P, D], BF16, tag="ysb")
                            state["ysb"] = ysb
                        ysb = state["ysb"]
                        yps = ps_aux.tile([P, WQ], F32, tag="aux")
                        nc.tensor.matmul(
                            yps[:],
                            ctxsb[:, qt, :], wo_sb[:, ds(oc * WQ, WQ)],
                            start=True, stop=True,
                        )
                        if eng == "scalar" or (eng == "split" and oc == 0):
                            nc.scalar.copy(ysb[:, ds(oc * WQ, WQ)], yps[:])
                        else:
                            nc.vector.tensor_copy(ysb[:, ds(oc * WQ, WQ)], yps[:])
                        nc.sync.dma_start(
                            y.ap()[ds(w * WQ + qt * P, P), ds(oc * WQ, WQ)],
                            ysb[:, ds(oc * WQ, WQ)])
                    return f
                return mk(0), mk(1)

            # ctx emission runs a few k-tiles behind the score/exp stream and
            # the score matmul for step i+1 is emitted BEFORE the exp for
            # step i, so the next exp's input is always already in the PE
            # queue ahead of the ctx burst (ScalarE never waits on scores).
            pending = deque()  # entries: (w, emit_fn, kt, pt)
            wstate = {}        # w -> (ctx_ps, ctxn, ctxsb, rcp)
            sts = {}           # (w, kt) -> st tile

            def get_wstate(w):
                if w not in wstate:
                    ctx_a = ps_ctx.tile([P, KTW, HD + 1], F32, tag="ctx0")
                    ctx_b = ps_ctx.tile([P, KTW, HD + 1], F32, tag="ctx1")
                    # explicit zero of the accumulator banks: a framework-
                    # visible write ordered after the previous window's norm
                    # reads (the matmuls below accumulate with start=False,
                    # so no bank-wide pending-zero side effect races ahead)
                    nc.vector.memset(ctx_a[:], 0.0)
                    nc.vector.memset(ctx_b[:], 0.0)
                    ctxn = ctxpool.tile([P, KTW, 2, HD], BF16, tag="ctxn")
                    ctxsb = ctxpool.tile([P, KTW, P], BF16, tag="ctxsb")
                    rcp = ctxpool.tile([P, 2 * KTW], F32, tag="rcp")
                    wstate[w] = ([ctx_a, ctx_b], ctxn, ctxsb, rcp)
                return wstate[w]

            def make_emit_ctx(w):
                ctx_ps, ctxn, ctxsb, rcp = get_wstate(w)

                def emit_ctx(kt, pt):
                    jo = kt - KTW * w
                    if jo >= 0:
                        # diagonal block: multiplicative causal mask on the
                        # post-exp P tile (bf16); only q-tile jo is partial
                        meng = nc.gpsimd if w == NW - 1 else nc.vector
                        for h in (0, 1):
                            meng.tensor_mul(
                                pt[:, h, ds(P * jo, P)],
                                pt[:, h, ds(P * jo, P)], tri_sb[:],
                            )
                    # ONE psum accumulation group per head-bank per window:
                    # start on the very first matmul (its start marks the
                    # whole 2KB bank pending-zero, so every q-tile region
                    # starts from zero), stop on the very last; interior
                    # matmuls accumulate (first touch of a pending byte
                    # overwrites).
                    nkt = KTW * (w + 1)
                    for h in (0, 1):
                        for qt in range(max(0, jo), KTW):
                            nc.tensor.matmul(
                                ctx_ps[h][:, qt, :],
                                pt[:, h, ds(qt * P, P)], V65[:, h, kt, :],
                                start=False,
                                stop=(kt == nkt - 1 and qt == KTW - 1),
                                skip_group_check=True,
                            )
                    if jo >= 0:
                        # accumulator (h, jo) just retired: queue its epilogue
                        qt = jo
                        lastw = w == NW - 1
                        last = lastw and qt == KTW - 1
                        bg_epi.append(t_norm(
                            ctx_ps, ctxn, rcp, 0, qt,
                            eng="scalar" if last else None))
                        bg_epi.append(t_norm(ctx_ps, ctxn, rcp, 1, qt))
                        bg_epi.append(t_ctrans(
                            ctxn, ctxsb, qt, eng="scalar" if last else None))
                        bg_epi.extend(t_outproj(
                            w, ctxsb, qt, eng="split" if lastw else None))
                return emit_ctx

            emitters = {}

            def emit_scores(w, kt):
                jo = kt - KTW * w
                soff = P * jo if jo > 0 else 0
                st = ps_st.tile([P, 2, WQ], F32, tag="st")
                for h in (0, 1):
                    ph = ds(HD * h, HD)
                    nc.tensor.matmul(
                        st[:, h, soff:WQ],
                        KT[ph, ds(kt * P, P)], QT[ph, ds(w * WQ + soff, WQ - soff)],
                        start=True, stop=True,
                        tile_position=(HD * h, 0),
                    )
                sts[(w, kt)] = st

            def run_attention():
                steps = [(w, kt) for w in range(NW) for kt in range(KTW * (w + 1))]
                # urgent-pop the rest of chunk 0's K/V projections BEFORE the
                # one-ahead score stream starts reading them (engine program
                # order must put writers before readers)
                pace(-1)
                scored = 0  # steps[0] scores were emitted in the prologue
                for i, (w, kt) in enumerate(steps):
                    if w not in emitters:
                        emitters[w] = make_emit_ctx(w)
                    nkt = KTW * (w + 1)
                    jo = kt - KTW * w
                    soff = P * jo if jo > 0 else 0
                    if scored < min(i + 1, len(steps) - 1):
                        scored += 1
                        emit_scores(*steps[scored])
                    st = sts.pop((w, kt))
                    pt = ptpool.tile([P, 2, WQ], BF16, tag="pt")
                    nc.scalar.activation(
                        pt[:, :, soff:WQ], st[:, :, soff:WQ],
                        mybir.ActivationFunctionType.Exp, scale=SCALE,
                    )
                    pending.append((w, emitters[w], kt, pt))
                    # drain carried ctx from the previous window first;
                    # near the end of the LAST window drain eagerly so the
                    # per-q-tile epilogues overlap the exp tail
                    stag = 0 if (kt >= nkt - 3 or w == NW - 1) else 4
                    drained = 0
                    while pending and pending[0][0] != w and drained < 2:
                        _, fn, *a_ = pending.popleft()
                        fn(*a_)
                        drained += 1
                    while (pending and pending[0][0] == w
                           and len(pending) > stag and drained < 4):
                        _, fn, *a_ = pending.popleft()
                        fn(*a_)
                        drained += 1
                    pace(i, epi=6 if w == NW - 1 else (4 if (jo < 1 and kt >= 3) else 1))

            # ---- software-pipelined emission ----
            # PE warm-up: dummy matmuls on (uninitialized) SBUF ramp the PE
            # p-state to full clock while the first x/weight DMAs land
            warm_mm = cpool.tile([P, WQ], BF16, tag="warm_mm")
            nc.vector.memset(warm_mm[:], 0.0)
            warm_ps = ps_st.tile([P, 2, WQ], F32, tag="st")
            NWARM = 9
            for i in range(NWARM):
                nc.tensor.matmul(
                    warm_ps[:, 0, :], warm_mm[:, 0:P], warm_mm[:],
                    start=(i == 0), stop=(i == NWARM - 1), skip_group_check=True,
                )
            # prologue: only what window 0's first scores need (Q chunk 0 and
            # the first k-tile column block of K); the rest rides in bg
            for _f in t_proj(0, wq_sb, wqc_sb, QT):
                _f()
            for _f in t_proj(0, wk_sb, wkc_sb, KT, 0, P):
                _f()
            emit_scores(0, 0)
            t_xt_dma(1)()
            t_xt_dma(2)()

            def SW(w):
                return 2 * w * (w + 1)  # step index of window w's first k-tile

            pre = []  # (deadline, fn) in dependency order

            ka, kb = t_proj(0, wk_sb, wkc_sb, KT, P, WQ, eng="scalar")
            pre += [(-2, ka), (-2, kb)]
            va, vb = t_proj(0, wv_sb, wvc_sb, VT, eng="scalar")
            pre += [(-1, va), (-1, vb)]
            for t in range(KTW):
                pre.append((max(-1, t - 2), t_vtrans(t)))
            qa, qb = t_proj(1, wq_sb, wqc_sb, QT)
            pre += [(SW(1) - 4, qa), (SW(1) - 3, qb)]
            ka, kb = t_proj(1, wk_sb, wkc_sb, KT)
            pre += [(SW(1) - 1, ka), (SW(1), kb)]
            va, vb = t_proj(1, wv_sb, wvc_sb, VT)
            pre += [(SW(1) + KTW - 10, va), (SW(1) + KTW - 9, vb)]
            for t in range(KTW, 2 * KTW):
                pre.append((SW(1) + t - 4, t_vtrans(t)))
            for c in range(2, NW):
                if c + 1 < NW:
                    pre.append((SW(c) - 14, t_xt_dma(c + 1)))
                qa, qb = t_proj(c, wq_sb, wqc_sb, QT)
                pre += [(SW(c) - 8, qa), (SW(c) - 7, qb)]
                ka, kb = t_proj(c, wk_sb, wkc_sb, KT)
                kd = SW(c) + KTW * c - (9 if c > 2 else 6)
                pre += [(kd, ka), (kd + 1, kb)]
                va, vb = t_proj(c, wv_sb, wvc_sb, VT)
                vd = SW(c) + KTW * c - (12 if c > 2 else 8)
                pre += [(vd, va), (vd + 1, vb)]
                for t in range(KTW * c, KTW * (c + 1)):
                    pre.append((SW(c) + t - 4, t_vtrans(t)))
            for seq, (d, fn) in enumerate(pre):
                bg_pre.append((d, seq, fn))
            bg_pre.sort()

            run_attention()

            # epilogue: flush the ctx backlog (which queues the last window's
            # per-q-tile epilogues), then drain both queues
            while pending:
                _, fn, *a_ = pending.popleft()
                fn(*a_)
            while bg_pre:
                bg_pre.pop(0)[2]()
            while bg_epi:
                bg_epi.popleft()()

    nc.compile()
    return nc


def _get_nc():
    global _CACHED_NC
    if _CACHED_NC is None:
        _CACHED_NC = _build()
    return _CACHED_NC


def kernel(x, Wq, Wk, Wv, Wo, bo):
    import ml_dtypes

    x = np.asarray(x, dtype=np.float32)
    Wq = np.asarray(Wq, dtype=np.float32)
    Wk = np.asarray(Wk, dtype=np.float32)
    Wv = np.asarray(Wv, dtype=np.float32)
    Wo = np.asarray(Wo, dtype=np.float32)
    bo = np.asarray(bo, dtype=np.float32)

    bf = ml_dtypes.bfloat16
    e4 = ml_dtypes.float8_e4m3
    xT = np.ascontiguousarray(x.reshape(S, D).T)
    x8 = xT.astype(e4)
    xr = (xT - x8.astype(np.float32)).astype(e4)
    # [NW, P, IO, 2, WQ]: partition-first, contiguous per partition row
    def xprep(a):
        return a.reshape(IO, P, NW, WQ).transpose(2, 1, 0, 3)
    xb = np.ascontiguousarray(
        np.stack([xprep(x8), xprep(xr)], axis=3))
    col = np.arange(P)
    # tri[k, q] = 1 where q >= k (valid), 0 above the diagonal
    tri = (col[None, :] >= col[:, None]).astype(bf)
    ident = np.eye(P, dtype=np.float32).astype(bf)

    def wsplit(W, dsl):
        wT = np.ascontiguousarray(W[dsl, :].T) * WSC
        w8 = wT.astype(e4)
        wr = (wT - w8.astype(np.float32)).astype(e4)
        # [P, IO, P] / [P, IO, 2, P]: partition-first
        w8p = w8.reshape(IO, P, P).transpose(1, 0, 2)
        wrp = wr.reshape(IO, P, P).transpose(1, 0, 2)
        wc = np.ascontiguousarray(np.stack([wrp, w8p], axis=2))
        return np.ascontiguousarray(w8p), wc

    in_maps = []
    for c in range(NC):
        dsl = slice(P * c, P * (c + 1))
        q8, qc = wsplit(Wq, dsl)
        k8, kc = wsplit(Wk, dsl)
        v8, vc = wsplit(Wv, dsl)
        in_maps.append({
            "xb": xb,
            "wq8": q8, "wqc": qc,
            "wk8": k8, "wkc": kc,
            "wv8": v8, "wvc": vc,
            "wo": np.ascontiguousarray(Wo[:, dsl].T).astype(bf),
            "tri": tri,
            "ident": ident,
        })

    nc = _get_nc()
    res = run_bass_kernel_spmd(nc, in_maps, core_ids=list(range(NC)))
    out = np.zeros((S, D), dtype=np.float32)
    for c in range(NC):
        out += res.results[c]["y"].astype(np.float32)
    out += bo[None, :]
    return out.reshape(1, S, D)
